# revision 4
# baseline (speedup 1.0000x reference)
"""Trainium2 Bass kernel for nn_AttentionModule (gnn_message_passing), v2.

Takes FULL inputs, shards batch dim across 8 NeuronCores (pure data
parallel), runs a hand-written Bass/Tile kernel per core, gathers the
full output.

v2 design (vs v1 baseline):
  - 4 load + 4 store DMAs per core (2.2MB each, k-natural order) instead
    of 34 x 512KB.
  - Two pipelined compute chunks of 256 batches each; load/compute/store
    overlap across chunks.
  - bf16 everywhere after the PE transposes (transpose converts
    fp32r -> bf16 at the PSUM write), halving DVE copy cost.
  - sigmoid(z) computed as 0.5*tanh(z/2)+0.5 with the 1/2 folded into
    the attention weights, so every Act-engine op shares one activation
    table (silu_and_others contains both Silu and Tanh) - this kills the
    ~18us act-table thrashing seen in the v1 trace.
  - keypoint pairs / target pairs stacked on 128 partitions so Act ops
    run on [128, 512] tiles instead of [64, 512].
  - gating multiplies in-place into the x tile; stores go straight from
    it.

Self-contained: all shapes/constants hardcoded.
"""

import numpy as np
import ml_dtypes

import concourse.bass as bass
import concourse.bacc as bacc_mod
import concourse.tile as tile
from concourse import mybir
from concourse.bass_utils import run_bass_kernel_spmd

# ---------------- problem constants (hardcoded) ----------------
B, K, C, CI = 4096, 17, 256, 64
NCORES = 8
BC = B // NCORES            # 512 batches per core
R = BC * K                  # 8704 rows per core
NT = BC // 128              # 4 partition-blocks of 128 batches
GROUPS = [[0, 1, 2, 3, 4], [5, 7, 9], [6, 8, 10], [11, 13, 15], [12, 14, 16]]
G = len(GROUPS)
KORDER = [k for g in GROUPS for k in g]          # slot -> original keypoint
SOFF = [0, 5, 8, 11, 14, 17]                     # group offsets in slot space
BN_EPS = 1e-5

F32 = mybir.dt.float32
F32R = mybir.dt.float32r
BF16 = mybir.dt.bfloat16
AFT = mybir.ActivationFunctionType
ALU = mybir.AluOpType


def build_nc(repeat=1):
    nc = bacc_mod.Bacc()
    x_h = nc.declare_dram_parameter("x", [R, C], F32R, isOutput=False)
    w1_h = nc.declare_dram_parameter("w1", [128, 2 * CI], BF16, isOutput=False)
    b1_h = nc.declare_dram_parameter("b1", [128, 1], F32, isOutput=False)
    wpe_h = nc.declare_dram_parameter("wpe", [CI, 10 * CI], BF16, isOutput=False)
    b2_h = nc.declare_dram_parameter("b2", [128, 1], F32, isOutput=False)
    wag_h = nc.declare_dram_parameter("wag", [CI + 1, C], BF16, isOutput=False)
    id_h = nc.declare_dram_parameter("ident", [128, 128], F32R, isOutput=False)
    ones_h = nc.declare_dram_parameter("ones", [1, G * BC], BF16, isOutput=False)
    out_h = nc.declare_dram_parameter("out", [R, C], F32, isOutput=True)

    # row r of x = b*K + k with b = t*128 + p  ->  view [t, p, k, c]
    x_r = x_h[:].rearrange("(t p k) c -> t p k c", t=NT, p=128, k=K)
    out_r = out_h[:].rearrange("(t p k) c -> t p k c", t=NT, p=128, k=K)

    # edge lists per target, matching reference EDGES order is irrelevant:
    # agg[i] = sum over j != i of silu2(Wp_i^T s_i + W2_j^T s_j + b2)
    JS = [[j for j in range(G) if j != i] for i in range(G)]

    import contextlib
    with tile.TileContext(nc) as tc:
        rep_ctx = (
            tc.For_i(0, repeat, 1, hint_engines=(mybir.EngineType.PE,))
            if repeat > 1 else contextlib.nullcontext()
        )
        with (
            tc.tile_pool(name="consts", bufs=1) as consts,
            tc.tile_pool(name="xin", bufs=4) as xin,
            tc.tile_pool(name="outp", bufs=3) as outp,
            tc.tile_pool(name="xts", bufs=4) as xtsp,
            tc.tile_pool(name="xds", bufs=2) as xdsp,
            tc.tile_pool(name="sums", bufs=2) as sumsp,
            tc.tile_pool(name="pes", bufs=2) as pesp,
            tc.tile_pool(name="aggs", bufs=2) as aggp,
            tc.tile_pool(name="atts", bufs=2) as attp,
            tc.tile_pool(name="pst", bufs=2, space="PSUM") as pstp,
            tc.tile_pool(name="pxd", bufs=2, space="PSUM") as pxdp,
            tc.tile_pool(name="ppe", bufs=2, space="PSUM") as ppep,
            rep_ctx,
        ):
            # ---- x loads first (sync ring), consts on the scalar ring ----
            # per t-block, two half-loads (kp 0:9 and 9:17) so transposes can
            # start after only the first halves arrive; order t0a,t1a,t0b,t1b
            xt = {}
            for ich in range(NT // 2):
                ts_ = (2 * ich, 2 * ich + 1)
                for t in ts_:
                    xt[t] = xin.tile([128, K * C], F32R, name="xtile")
                for lo, hi in ((0, 9), (9, K)):
                    for t in ts_:
                        nc.sync.dma_start(
                            out=xt[t].rearrange("p (k c) -> p k c", c=C)[:, lo:hi, :],
                            in_=x_r[t][:, lo:hi, :],
                        )

            # ---- constants (scalar HWDGE ring; ident first) ----
            ident_r = consts.tile([128, 128], F32R)
            nc.scalar.dma_start(out=ident_r, in_=id_h[:])
            w1_sb = consts.tile([128, 2 * CI], BF16)
            nc.scalar.dma_start(out=w1_sb, in_=w1_h[:])
            b1_sb = consts.tile([128, 1], F32)
            nc.scalar.dma_start(out=b1_sb, in_=b1_h[:])
            # wpe / wag duplicated into both partition halves: the edge and
            # attention matmuls contract over 128 partitions, summing the
            # stacked even/odd partials for free
            wpe_sb = consts.tile([128, 10 * CI], BF16)
            nc.scalar.dma_start(out=wpe_sb[0:CI, :], in_=wpe_h[:])
            nc.scalar.dma_start(out=wpe_sb[CI:128, :], in_=wpe_h[:])
            b2_sb = consts.tile([128, 1], F32)
            nc.scalar.dma_start(out=b2_sb, in_=b2_h[:])
            wag_sb = consts.tile([128, C], BF16)
            nc.scalar.dma_start(out=wag_sb[0:CI, :], in_=wag_h[0:CI, :])
            nc.scalar.dma_start(out=wag_sb[CI:128, :], in_=wag_h[0:CI, :])
            wagb_sb = consts.tile([CI + 1, C], BF16)
            nc.scalar.dma_start(out=wagb_sb[0:1, :], in_=wag_h[CI:CI + 1, :])
            nc.scalar.dma_start(out=wagb_sb[CI:CI + 1, :], in_=wag_h[CI:CI + 1, :])
            ones_sb = consts.tile([CI + 1, 128], BF16)
            nc.scalar.dma_start(out=ones_sb[0:1, :], in_=ones_h[0:1, 0:128])
            nc.scalar.dma_start(out=ones_sb[CI:CI + 1, :], in_=ones_h[0:1, 0:128])

            ncopy = [0]  # alternates PSUM->SBUF copies between DVE and Act

            # pairs needing only kp<9 first (their half-a loads land first)
            PAIR_ORDER = [0, 1, 2, 4, 3, 5, 6, 7, 8]

            # per-chunk state tiles (pools hold 2 bufs: both chunks live)
            state = {}

            def front(ich):
                """transpose + down-proj + silu1 + group sums for one chunk.

                slot s -> original k = KORDER[s]; pair p9 = slots (2p9, 2p9+1)
                xd_sb column block of slot s = (s//2)*256, row half 64*(s%2)
                """
                ts = (2 * ich, 2 * ich + 1)
                xd_sb = xdsp.tile([128, 9 * 256], BF16)
                sums_sb = sumsp.tile([128, G * 256], BF16)
                state[ich] = {"xd": xd_sb, "sums": sums_sb}
                xdq_of = {}
                pairs_done = set()

                def xd_ap(s):
                    return xd_sb[64 * (s % 2):64 * (s % 2) + 64,
                                 (s // 2) * 256:(s // 2) * 256 + 256]

                for p9 in PAIR_ORDER:
                    q = p9 // 2
                    if q not in xdq_of:
                        if q < 4:
                            xdq_of[q] = pxdp.tile([128, 512], F32, name="xdq")
                        else:
                            xdq_of[q] = pxdp.tile([64, 256], F32, name="xdq4",
                                                  bufs=1)
                    xdq = xdq_of[q]
                    lp = p9 % 2
                    slots = [2 * p9, 2 * p9 + 1] if p9 < 8 else [16]
                    ns = len(slots)
                    xts = []
                    for ch in range(2):
                        xts_t = xtsp.tile([128, 256 * ns], BF16, name="xts")
                        xts_v = xts_t.rearrange("p (s u) -> p s u", s=ns)
                        for ti, t in enumerate(ts):
                            pst = pstp.tile([128, 128 * ns], F32R, name="pst")
                            for si, s in enumerate(slots):
                                k = KORDER[s]
                                nc.tensor.transpose(
                                    out=pst[:, si * 128:(si + 1) * 128],
                                    in_=xt[t][:, k * C + ch * 128: k * C + ch * 128 + 128],
                                    identity=ident_r,
                                )
                            # fp32r -> bf16 conversion happens in the copy
                            dst = xts_v[:, :, ti * 128:(ti + 1) * 128]
                            src = pst.rearrange("p (s u) -> p s u", s=ns)
                            if ncopy[0] % 2 == 0:
                                nc.vector.tensor_copy(out=dst, in_=src)
                            else:
                                nc.scalar.copy(out=dst, in_=src.bitcast(F32))
                            ncopy[0] += 1
                        xts.append(xts_t)
                    for si, s in enumerate(slots):
                        for ch in range(2):
                            nc.tensor.matmul(
                                out=xdq[64 * (s % 2):64 * (s % 2) + 64,
                                        lp * 256:lp * 256 + 256],
                                lhsT=w1_sb[:, ch * CI:(ch + 1) * CI],
                                rhs=xts[ch][:, si * 256:si * 256 + 256],
                                start=(ch == 0), stop=(ch == 1),
                                skip_group_check=True,
                            )
                    # silu1 fires once both pairs of the bank are done
                    pairs_done.add(p9)
                    bank_pairs = [2 * q, 2 * q + 1] if q < 4 else [8]
                    if all(p in pairs_done for p in bank_pairs):
                        if q < 4:
                            nc.scalar.activation(
                                out=xd_sb[:, q * 512:(q + 1) * 512], in_=xdq,
                                func=AFT.Silu, bias=b1_sb,
                            )
                        else:
                            nc.scalar.activation(
                                out=xd_sb[0:64, 2048:2304], in_=xdq,
                                func=AFT.Silu, bias=b1_sb[0:64],
                            )

                # group sums at chunk width, kept as stacked even/odd-slot
                # partials (rows 0:64 / 64:128); partition-aligned adds only.
                # The downstream matmuls contract over all 128 partitions.
                for g in range(G):
                    slots = list(range(SOFF[g], SOFF[g + 1]))
                    for half in range(2):
                        hs = [s for s in slots if s % 2 == half]
                        sl = sums_sb[64 * half:64 * half + 64,
                                     g * 256:(g + 1) * 256]
                        if len(hs) == 1:
                            nc.vector.tensor_copy(out=sl, in_=xd_ap(hs[0]))
                        else:
                            nc.vector.tensor_add(out=sl, in0=xd_ap(hs[0]),
                                                 in1=xd_ap(hs[1]))
                            for s in hs[2:]:
                                nc.vector.tensor_add(out=sl, in0=sl,
                                                     in1=xd_ap(s))

            def back(ich, th):
                """edges + agg + att + gating + store for one t-block."""
                t = 2 * ich + th
                sums_sb = state[ich]["sums"]
                if th == 0:
                    state[ich]["pe"] = pesp.tile([128, 2 * 3 * 512], BF16,
                                                 name="pe_sb")
                    state[ich]["agg"] = aggp.tile([128, G * 256], BF16,
                                                  name="agg_sb")
                    state[ich]["att"] = attp.tile([128, 2 * G * C], BF16,
                                                  name="att_sb")
                pe_sb = state[ich]["pe"]
                agg_sb = state[ich]["agg"]
                att_sb = state[ich]["att"]

                def pe_ap(th_, tgt, e):
                    col = th_ * 1536 + (tgt // 2) * 512 + e * 128
                    rh = 64 * (tgt % 2)
                    return pe_sb[rh:rh + 64, col:col + 128]

                if True:
                    # edge conv: target pairs stacked on partitions;
                    # one PSUM bank per target pair = 2 tgts x 4 edges x 128
                    for pi, tgts in enumerate([(0, 1), (2, 3), (4,)]):
                        rows = 64 * len(tgts)
                        pep = ppep.tile([rows, 512], F32)
                        for rh, tgt in zip((0, 64), tgts):
                            for e in range(4):
                                j = JS[tgt][e]
                                outap = pep[rh:rh + 64, e * 128:e * 128 + 128]
                                nc.tensor.matmul(
                                    out=outap,
                                    lhsT=wpe_sb[:, tgt * CI:(tgt + 1) * CI],
                                    rhs=sums_sb[:, tgt * 256 + th * 128:
                                                tgt * 256 + th * 128 + 128],
                                    start=True, stop=False,
                                    skip_group_check=True,
                                )
                                nc.tensor.matmul(
                                    out=outap,
                                    lhsT=wpe_sb[:, (G + j) * CI:(G + j + 1) * CI],
                                    rhs=sums_sb[:, j * 256 + th * 128:
                                                j * 256 + th * 128 + 128],
                                    start=False, stop=True,
                                    skip_group_check=True,
                                )
                        nc.scalar.activation(
                            out=pe_sb[0:rows, th * 1536 + pi * 512:
                                      th * 1536 + (pi + 1) * 512],
                            in_=pep, func=AFT.Silu, bias=b2_sb[0:rows],
                        )

                    # scatter-add over target nodes; agg slab for tgt lives in
                    # the same partition half as its pe slabs (aligned adds)
                    for tgt in range(G):
                        rh = 64 * (tgt % 2)
                        sl = agg_sb[rh:rh + 64,
                                    tgt * 256 + th * 128:tgt * 256 + th * 128 + 128]
                        nc.vector.tensor_add(out=sl, in0=pe_ap(th, tgt, 0),
                                             in1=pe_ap(th, tgt, 1))
                        nc.vector.tensor_add(out=sl, in0=sl, in1=pe_ap(th, tgt, 2))
                        nc.vector.tensor_add(out=sl, in0=sl, in1=pe_ap(th, tgt, 3))

                    # attention: att' = 0.5*tanh(z/2)+0.5, z/2 folded in wag
                    for gp in ((0, 1), (2, 3), (4,)):
                        gl = 256 * len(gp)
                        patt = ppep.tile([128, 512], F32, name="pep")
                        patt = patt[:, 0:gl]
                        for gi, g in enumerate(gp):
                            rh = 64 * (g % 2)
                            outap = patt[:, gi * 256:gi * 256 + 256]
                            nc.tensor.matmul(
                                out=outap,
                                lhsT=agg_sb[rh:rh + 64,
                                            g * 256 + th * 128:g * 256 + th * 128 + 128],
                                rhs=wag_sb[rh:rh + 64, :],
                                start=True, stop=False,
                                skip_group_check=True,
                            )
                            nc.tensor.matmul(
                                out=outap,
                                lhsT=ones_sb[rh:rh + 1, :],
                                rhs=wagb_sb[rh:rh + 1, :],
                                start=False, stop=True,
                                skip_group_check=True,
                            )
                        nc.scalar.activation(
                            out=att_sb[:, th * G * C + gp[0] * 256:
                                       th * G * C + gp[0] * 256 + gl],
                            in_=patt, func=AFT.Tanh,
                        )
                        # (tanh+1)*0.5 in place, per group-pair so gating
                        # of early groups starts sooner
                        sl = att_sb[:, th * G * C + gp[0] * 256:
                                    th * G * C + gp[0] * 256 + gl]
                        nc.vector.tensor_scalar(
                            out=sl, in0=sl, scalar1=1.0, scalar2=0.5,
                            op0=ALU.add, op1=ALU.mult,
                        )

                    # gating into a fresh output tile + per-group store
                    xv = xt[t].bitcast(F32).rearrange("p (k c) -> p k c", c=C)
                    ot = outp.tile([128, K * C], F32, name="otile")
                    ov = ot.rearrange("p (k c) -> p k c", c=C)
                    for g in range(G):
                        ks = GROUPS[g]
                        step = ks[1] - ks[0] if len(ks) > 1 else 1
                        xg = xv[:, ks[0]:ks[-1] + 1:step, :]
                        og = ov[:, ks[0]:ks[-1] + 1:step, :]
                        av = att_sb[:, th * G * C + g * 256:th * G * C + (g + 1) * 256]
                        av = av.rearrange("p (u c) -> p u c", u=1).broadcast_to(
                            (128, len(ks), C))
                        eng = nc.vector if g < 2 else nc.gpsimd
                        eng.tensor_mul(out=og, in0=xg, in1=av)
                        nc.sync.dma_start(
                            out=out_r[t][:, ks[0]:ks[-1] + 1:step, :],
                            in_=og,
                        )

            # software-pipelined emission: chunk1's front fills the engine
            # gaps while chunk0's back half drains toward its stores
            front(0)
            back(0, 0)
            front(1)
            back(0, 1)
            back(1, 0)
            back(1, 1)

    nc.compile()
    return nc


def _prep_weights(W_down, b_down, bn1_scale, bn1_bias, bn1_mean, bn1_var,
                  W_conv, bn2_scale, bn2_bias, bn2_mean, bn2_var, W_agg, b_agg):
    f64 = np.float64
    bf16 = ml_dtypes.bfloat16
    a1 = bn1_scale.astype(f64) / np.sqrt(bn1_var.astype(f64) + BN_EPS)
    W1f = W_down.astype(f64) * a1[None, :]                      # [256, 64]
    b1f = (b_down.astype(f64) - bn1_mean) * a1 + bn1_bias       # [64]

    a2 = bn2_scale.astype(f64) / np.sqrt(bn2_var.astype(f64) + BN_EPS)
    Wc = W_conv.astype(f64) * a2[:, None]                       # [64, 128]
    b2f = bn2_bias.astype(f64) - bn2_mean * a2                  # [64]
    W1, W2 = Wc[:, :CI], Wc[:, CI:]
    Wp = W1 - W2

    GS = [len(g) for g in GROUPS]
    # w1 sbuf layout: [128, 2*CI], col-block ch = W1f[ch*128:(ch+1)*128, :]
    w1 = np.concatenate([W1f[:128, :], W1f[128:, :]], axis=1).astype(bf16)
    b1 = np.tile(b1f.reshape(CI, 1), (2, 1)).astype(np.float32)  # [128, 1]

    # wpe: [64, 10*64]: blocks 0..4 = Wp.T/|g_i|, 5..9 = W2.T/|g_j|
    blocks = [Wp.T / GS[i] for i in range(G)] + [W2.T / GS[j] for j in range(G)]
    wpe = np.concatenate(blocks, axis=1).astype(bf16)
    b2 = np.tile(b2f.reshape(CI, 1), (2, 1)).astype(np.float32)

    # wag carries the 1/2 of sigmoid(z) = 0.5*tanh(z/2)+0.5
    wag = (0.5 * np.concatenate(
        [W_agg.astype(f64), b_agg.astype(f64)[None, :]], axis=0
    )).astype(bf16)                                             # [65, 256]
    return w1, b1, wpe, b2, wag


_NC_CACHE = {}


def _run(inputs, trace=False, trace_kwargs=None):
    x = np.ascontiguousarray(np.asarray(inputs["x_bk_c"], dtype=np.float32))
    assert x.shape == (B * K, C), x.shape
    w1, b1, wpe, b2, wag = _prep_weights(
        np.asarray(inputs["W_down"]), np.asarray(inputs["b_down"]),
        np.asarray(inputs["bn1_scale"]), np.asarray(inputs["bn1_bias"]),
        np.asarray(inputs["bn1_mean"]), np.asarray(inputs["bn1_var"]),
        np.asarray(inputs["W_conv"]),
        np.asarray(inputs["bn2_scale"]), np.asarray(inputs["bn2_bias"]),
        np.asarray(inputs["bn2_mean"]), np.asarray(inputs["bn2_var"]),
        np.asarray(inputs["W_agg"]), np.asarray(inputs["b_agg"]),
    )

    if "nc" not in _NC_CACHE:
        _NC_CACHE["nc"] = build_nc()
    nc = _NC_CACHE["nc"]

    in_maps = []
    for c in range(NCORES):
        in_maps.append({
            "x": np.ascontiguousarray(x[c * R:(c + 1) * R]),
            "w1": w1, "b1": b1, "wpe": wpe, "b2": b2, "wag": wag,
            "ident": np.eye(128, dtype=np.float32),
            "ones": np.ones((1, G * BC), dtype=ml_dtypes.bfloat16),
        })
    kw = {}
    if trace:
        kw["trace"] = True
        if trace_kwargs:
            kw["trace_kwargs"] = trace_kwargs
    res = run_bass_kernel_spmd(nc, in_maps, core_ids=list(range(NCORES)), **kw)
    out = np.concatenate([r["out"] for r in res.results], axis=0)
    return out, res


def kernel(**inputs) -> np.ndarray:
    out, _ = _run(inputs)
    return out


# revision 5
# speedup vs baseline: 1.3131x; 1.3131x over previous
"""Trainium2 Bass kernel for nn_AttentionModule (gnn_message_passing), v2.

Takes FULL inputs, shards batch dim across 8 NeuronCores (pure data
parallel), runs a hand-written Bass/Tile kernel per core, gathers the
full output.

v2 design (vs v1 baseline):
  - 4 load + 4 store DMAs per core (2.2MB each, k-natural order) instead
    of 34 x 512KB.
  - Two pipelined compute chunks of 256 batches each; load/compute/store
    overlap across chunks.
  - bf16 everywhere after the PE transposes (transpose converts
    fp32r -> bf16 at the PSUM write), halving DVE copy cost.
  - sigmoid(z) computed as 0.5*tanh(z/2)+0.5 with the 1/2 folded into
    the attention weights, so every Act-engine op shares one activation
    table (silu_and_others contains both Silu and Tanh) - this kills the
    ~18us act-table thrashing seen in the v1 trace.
  - keypoint pairs / target pairs stacked on 128 partitions so Act ops
    run on [128, 512] tiles instead of [64, 512].
  - gating multiplies in-place into the x tile; stores go straight from
    it.

Self-contained: all shapes/constants hardcoded.
"""

import numpy as np
import ml_dtypes

import concourse.bass as bass
import concourse.bacc as bacc_mod
import concourse.tile as tile
from concourse import mybir
from concourse.bass_utils import run_bass_kernel_spmd

# ---------------- problem constants (hardcoded) ----------------
B, K, C, CI = 4096, 17, 256, 64
NCORES = 8
BC = B // NCORES            # 512 batches per core
R = BC * K                  # 8704 rows per core
NT = BC // 128              # 4 partition-blocks of 128 batches
GROUPS = [[0, 1, 2, 3, 4], [5, 7, 9], [6, 8, 10], [11, 13, 15], [12, 14, 16]]
G = len(GROUPS)
KORDER = [k for g in GROUPS for k in g]          # slot -> original keypoint
SOFF = [0, 5, 8, 11, 14, 17]                     # group offsets in slot space
BN_EPS = 1e-5

F32 = mybir.dt.float32
F32R = mybir.dt.float32r
BF16 = mybir.dt.bfloat16
AFT = mybir.ActivationFunctionType
ALU = mybir.AluOpType


def build_nc(repeat=1):
    nc = bacc_mod.Bacc()
    x_h = nc.declare_dram_parameter("x", [R, C], F32R, isOutput=False)
    w1_h = nc.declare_dram_parameter("w1", [128, 2 * CI], BF16, isOutput=False)
    b1_h = nc.declare_dram_parameter("b1", [128, 1], F32, isOutput=False)
    wpe_h = nc.declare_dram_parameter("wpe", [CI, 10 * CI], BF16, isOutput=False)
    b2_h = nc.declare_dram_parameter("b2", [128, 1], F32, isOutput=False)
    wag_h = nc.declare_dram_parameter("wag", [CI + 1, C], BF16, isOutput=False)
    id_h = nc.declare_dram_parameter("ident", [128, 128], F32R, isOutput=False)
    ones_h = nc.declare_dram_parameter("ones", [1, G * BC], BF16, isOutput=False)
    out_h = nc.declare_dram_parameter("out", [R, C], F32, isOutput=True)

    # row r of x = b*K + k with b = t*128 + p  ->  view [t, p, k, c]
    x_r = x_h[:].rearrange("(t p k) c -> t p k c", t=NT, p=128, k=K)
    out_r = out_h[:].rearrange("(t p k) c -> t p k c", t=NT, p=128, k=K)

    # edge lists per target, matching reference EDGES order is irrelevant:
    # agg[i] = sum over j != i of silu2(Wp_i^T s_i + W2_j^T s_j + b2)
    JS = [[j for j in range(G) if j != i] for i in range(G)]

    import contextlib
    with tile.TileContext(nc) as tc:
        rep_ctx = (
            tc.For_i(0, repeat, 1, hint_engines=(mybir.EngineType.PE,))
            if repeat > 1 else contextlib.nullcontext()
        )
        with (
            tc.tile_pool(name="consts", bufs=1) as consts,
            tc.tile_pool(name="xin", bufs=4) as xin,
            tc.tile_pool(name="outp", bufs=3) as outp,
            tc.tile_pool(name="xts", bufs=4) as xtsp,
            tc.tile_pool(name="xds", bufs=2) as xdsp,
            tc.tile_pool(name="sums", bufs=2) as sumsp,
            tc.tile_pool(name="pes", bufs=2) as pesp,
            tc.tile_pool(name="aggs", bufs=2) as aggp,
            tc.tile_pool(name="atts", bufs=2) as attp,
            tc.tile_pool(name="pst", bufs=2, space="PSUM") as pstp,
            tc.tile_pool(name="pxd", bufs=2, space="PSUM") as pxdp,
            tc.tile_pool(name="ppe", bufs=2, space="PSUM") as ppep,
            rep_ctx,
        ):
            # ---- x loads first (sync ring), consts on the scalar ring ----
            # per t-block, two half-loads (kp 0:9 and 9:17) so transposes can
            # start after only the first halves arrive; order t0a,t1a,t0b,t1b
            xt = {}
            for ich in range(NT // 2):
                ts_ = (2 * ich, 2 * ich + 1)
                for t in ts_:
                    xt[t] = xin.tile([128, K * C], F32R, name="xtile")
                for lo, hi in ((0, 9), (9, K)):
                    for t in ts_:
                        nc.sync.dma_start(
                            out=xt[t].rearrange("p (k c) -> p k c", c=C)[:, lo:hi, :],
                            in_=x_r[t][:, lo:hi, :],
                        )

            # ---- constants (scalar HWDGE ring; ident first) ----
            ident_r = consts.tile([128, 128], F32R)
            nc.scalar.dma_start(out=ident_r, in_=id_h[:])
            w1_sb = consts.tile([128, 2 * CI], BF16)
            nc.scalar.dma_start(out=w1_sb, in_=w1_h[:])
            b1_sb = consts.tile([128, 1], F32)
            nc.scalar.dma_start(out=b1_sb, in_=b1_h[:])
            # wpe / wag duplicated into both partition halves: the edge and
            # attention matmuls contract over 128 partitions, summing the
            # stacked even/odd partials for free
            wpe_sb = consts.tile([128, 10 * CI], BF16)
            nc.scalar.dma_start(out=wpe_sb[0:CI, :], in_=wpe_h[:])
            nc.scalar.dma_start(out=wpe_sb[CI:128, :], in_=wpe_h[:])
            b2_sb = consts.tile([128, 1], F32)
            nc.scalar.dma_start(out=b2_sb, in_=b2_h[:])
            wag_sb = consts.tile([128, C], BF16)
            nc.scalar.dma_start(out=wag_sb[0:CI, :], in_=wag_h[0:CI, :])
            nc.scalar.dma_start(out=wag_sb[CI:128, :], in_=wag_h[0:CI, :])
            wagb_sb = consts.tile([CI + 1, C], BF16)
            nc.scalar.dma_start(out=wagb_sb[0:1, :], in_=wag_h[CI:CI + 1, :])
            nc.scalar.dma_start(out=wagb_sb[CI:CI + 1, :], in_=wag_h[CI:CI + 1, :])
            ones_sb = consts.tile([CI + 1, 128], BF16)
            nc.scalar.dma_start(out=ones_sb[0:1, :], in_=ones_h[0:1, 0:128])
            nc.scalar.dma_start(out=ones_sb[CI:CI + 1, :], in_=ones_h[0:1, 0:128])

            ncopy = [0]  # alternates PSUM->SBUF copies between DVE and Act

            # pairs needing only kp<9 first (their half-a loads land first)
            PAIR_ORDER = [0, 1, 2, 4, 3, 5, 6, 7, 8]

            # per-chunk state tiles (pools hold 2 bufs: both chunks live)
            state = {}

            def front(ich):
                """transpose + down-proj + silu1 + group sums for one chunk.

                slot s -> original k = KORDER[s]; pair p9 = slots (2p9, 2p9+1)
                xd_sb column block of slot s = (s//2)*256, row half 64*(s%2)
                """
                ts = (2 * ich, 2 * ich + 1)
                xd_sb = xdsp.tile([128, 9 * 256], BF16)
                sums_sb = sumsp.tile([128, G * 256], BF16)
                state[ich] = {"xd": xd_sb, "sums": sums_sb}
                xdq_of = {}
                pairs_done = set()

                def xd_ap(s):
                    return xd_sb[64 * (s % 2):64 * (s % 2) + 64,
                                 (s // 2) * 256:(s // 2) * 256 + 256]

                for p9 in PAIR_ORDER:
                    q = p9 // 2
                    if q not in xdq_of:
                        if q < 4:
                            xdq_of[q] = pxdp.tile([128, 512], F32, name="xdq")
                        else:
                            xdq_of[q] = pxdp.tile([64, 256], F32, name="xdq4",
                                                  bufs=1)
                    xdq = xdq_of[q]
                    lp = p9 % 2
                    slots = [2 * p9, 2 * p9 + 1] if p9 < 8 else [16]
                    ns = len(slots)
                    # early pairs (waiting on the second half-load) transpose
                    # per t-block; late pairs have all data - one big copy
                    early = p9 in (0, 1, 2)
                    xts = []
                    for ch in range(2):
                        xts_t = xtsp.tile([128, 256 * ns], BF16, name="xts")
                        xts_v = xts_t.rearrange("p (s u) -> p s u", s=ns)
                        if early:
                            for ti, t in enumerate(ts):
                                pst = pstp.tile([128, 128 * ns], F32R, name="pst")
                                for si, s in enumerate(slots):
                                    k = KORDER[s]
                                    nc.tensor.transpose(
                                        out=pst[:, si * 128:(si + 1) * 128],
                                        in_=xt[t][:, k * C + ch * 128: k * C + ch * 128 + 128],
                                        identity=ident_r,
                                    )
                                # fp32r -> bf16 conversion happens in the copy
                                dst = xts_v[:, :, ti * 128:(ti + 1) * 128]
                                src = pst.rearrange("p (s u) -> p s u", s=ns)
                                if ncopy[0] % 2 == 0:
                                    nc.vector.tensor_copy(out=dst, in_=src)
                                else:
                                    nc.scalar.copy(out=dst, in_=src.bitcast(F32))
                                ncopy[0] += 1
                        else:
                            pst = pstp.tile([128, 256 * ns], F32R, name="pst")
                            for si, s in enumerate(slots):
                                k = KORDER[s]
                                for ti, t in enumerate(ts):
                                    nc.tensor.transpose(
                                        out=pst[:, (si * 2 + ti) * 128:(si * 2 + ti + 1) * 128],
                                        in_=xt[t][:, k * C + ch * 128: k * C + ch * 128 + 128],
                                        identity=ident_r,
                                    )
                            if ncopy[0] % 2 == 0:
                                nc.vector.tensor_copy(out=xts_t, in_=pst)
                            else:
                                nc.scalar.copy(out=xts_t, in_=pst.bitcast(F32))
                            ncopy[0] += 1
                        xts.append(xts_t)
                    for si, s in enumerate(slots):
                        for ch in range(2):
                            nc.tensor.matmul(
                                out=xdq[64 * (s % 2):64 * (s % 2) + 64,
                                        lp * 256:lp * 256 + 256],
                                lhsT=w1_sb[:, ch * CI:(ch + 1) * CI],
                                rhs=xts[ch][:, si * 256:si * 256 + 256],
                                start=(ch == 0), stop=(ch == 1),
                                skip_group_check=True,
                            )
                    # silu1 fires once both pairs of the bank are done
                    pairs_done.add(p9)
                    bank_pairs = [2 * q, 2 * q + 1] if q < 4 else [8]
                    if all(p in pairs_done for p in bank_pairs):
                        if q < 4:
                            nc.scalar.activation(
                                out=xd_sb[:, q * 512:(q + 1) * 512], in_=xdq,
                                func=AFT.Silu, bias=b1_sb,
                            )
                        else:
                            nc.scalar.activation(
                                out=xd_sb[0:64, 2048:2304], in_=xdq,
                                func=AFT.Silu, bias=b1_sb[0:64],
                            )

                # group sums at chunk width, kept as stacked even/odd-slot
                # partials (rows 0:64 / 64:128); partition-aligned adds only.
                # The downstream matmuls contract over all 128 partitions.
                for g in range(G):
                    slots = list(range(SOFF[g], SOFF[g + 1]))
                    for half in range(2):
                        hs = [s for s in slots if s % 2 == half]
                        sl = sums_sb[64 * half:64 * half + 64,
                                     g * 256:(g + 1) * 256]
                        if len(hs) == 1:
                            nc.vector.tensor_copy(out=sl, in_=xd_ap(hs[0]))
                        else:
                            nc.vector.tensor_add(out=sl, in0=xd_ap(hs[0]),
                                                 in1=xd_ap(hs[1]))
                            for s in hs[2:]:
                                nc.vector.tensor_add(out=sl, in0=sl,
                                                     in1=xd_ap(s))

            def back(ich, th):
                """edges/agg chunk-wide on th==0; att+gating+store per t."""
                t = 2 * ich + th
                sums_sb = state[ich]["sums"]
                if th == 0:
                    state[ich]["pe"] = pesp.tile([128, 6 * 512], BF16,
                                                 name="pe_sb")
                    state[ich]["agg"] = aggp.tile([128, 3 * 256], BF16,
                                                  name="agg_sb")
                    state[ich]["att"] = attp.tile([128, 2 * G * C], BF16,
                                                  name="att_sb")
                pe_sb = state[ich]["pe"]
                agg_sb = state[ich]["agg"]
                att_sb = state[ich]["att"]

                # pe slab col for (tgt, e), chunk-wide (256 cols per slab)
                def pe_col(tgt, e):
                    return ((tgt // 2) * 2 + e // 2) * 512 + (e % 2) * 256

                if th == 0:
                    # edge conv at chunk width: bank (tgt-pair, e-pair) =
                    # [2 tgts stacked, 2 edges x 256]
                    for tp, tgts in enumerate([(0, 1), (2, 3), (4,)]):
                        rows = 64 * len(tgts)
                        for ep in range(2):
                            pep = ppep.tile([rows, 512], F32, name="pep")
                            for rh, tgt in zip((0, 64), tgts):
                                for el in range(2):
                                    e = ep * 2 + el
                                    j = JS[tgt][e]
                                    outap = pep[rh:rh + 64, el * 256:el * 256 + 256]
                                    nc.tensor.matmul(
                                        out=outap,
                                        lhsT=wpe_sb[:, tgt * CI:(tgt + 1) * CI],
                                        rhs=sums_sb[:, tgt * 256:(tgt + 1) * 256],
                                        start=True, stop=False,
                                        skip_group_check=True,
                                    )
                                    nc.tensor.matmul(
                                        out=outap,
                                        lhsT=wpe_sb[:, (G + j) * CI:(G + j + 1) * CI],
                                        rhs=sums_sb[:, j * 256:(j + 1) * 256],
                                        start=False, stop=True,
                                        skip_group_check=True,
                                    )
                            nc.scalar.activation(
                                out=pe_sb[0:rows, (tp * 2 + ep) * 512:
                                          (tp * 2 + ep + 1) * 512],
                                in_=pep, func=AFT.Silu, bias=b2_sb[0:rows],
                            )

                    # scatter-add, target pairs stacked: agg[rows of tgt%2,
                    # (tgt//2)*256] = sum_e pe(tgt, e); adds cover both tgts
                    for tp, tgts in enumerate([(0, 1), (2, 3), (4,)]):
                        rows = 64 * len(tgts)
                        sl = agg_sb[0:rows, tp * 256:(tp + 1) * 256]
                        nc.vector.tensor_add(
                            out=sl,
                            in0=pe_sb[0:rows, pe_col(tgts[0], 0):pe_col(tgts[0], 0) + 256],
                            in1=pe_sb[0:rows, pe_col(tgts[0], 1):pe_col(tgts[0], 1) + 256])
                        nc.vector.tensor_add(
                            out=sl, in0=sl,
                            in1=pe_sb[0:rows, pe_col(tgts[0], 2):pe_col(tgts[0], 2) + 256])
                        nc.vector.tensor_add(
                            out=sl, in0=sl,
                            in1=pe_sb[0:rows, pe_col(tgts[0], 3):pe_col(tgts[0], 3) + 256])

                # attention: att' = 0.5*tanh(z/2)+0.5, z/2 folded in wag
                for gp in ((0, 1), (2, 3), (4,)):
                    gl = 256 * len(gp)
                    patt = ppep.tile([128, 512], F32, name="pep")
                    patt = patt[:, 0:gl]
                    for gi, g in enumerate(gp):
                        rh = 64 * (g % 2)
                        outap = patt[:, gi * 256:gi * 256 + 256]
                        nc.tensor.matmul(
                            out=outap,
                            lhsT=agg_sb[rh:rh + 64,
                                        (g // 2) * 256 + th * 128:
                                        (g // 2) * 256 + th * 128 + 128],
                            rhs=wag_sb[rh:rh + 64, :],
                            start=True, stop=False,
                            skip_group_check=True,
                        )
                        nc.tensor.matmul(
                            out=outap,
                            lhsT=ones_sb[rh:rh + 1, :],
                            rhs=wagb_sb[rh:rh + 1, :],
                            start=False, stop=True,
                            skip_group_check=True,
                        )
                    nc.scalar.activation(
                        out=att_sb[:, th * G * C + gp[0] * 256:
                                   th * G * C + gp[0] * 256 + gl],
                        in_=patt, func=AFT.Tanh,
                    )
                # (tanh+1)*0.5 in place, per t-block
                sl = att_sb[:, th * G * C:(th + 1) * G * C]
                nc.vector.tensor_scalar(
                    out=sl, in0=sl, scalar1=1.0, scalar2=0.5,
                    op0=ALU.add, op1=ALU.mult,
                )

                # gating into a fresh output tile + one store per t-block
                xv = xt[t].bitcast(F32).rearrange("p (k c) -> p k c", c=C)
                ot = outp.tile([128, K * C], F32, name="otile")
                ov = ot.rearrange("p (k c) -> p k c", c=C)
                for g in range(G):
                    ks = GROUPS[g]
                    step = ks[1] - ks[0] if len(ks) > 1 else 1
                    xg = xv[:, ks[0]:ks[-1] + 1:step, :]
                    og = ov[:, ks[0]:ks[-1] + 1:step, :]
                    av = att_sb[:, th * G * C + g * 256:th * G * C + (g + 1) * 256]
                    av = av.rearrange("p (u c) -> p u c", u=1).broadcast_to(
                        (128, len(ks), C))
                    eng = nc.vector if g < 2 else nc.gpsimd
                    eng.tensor_mul(out=og, in0=xg, in1=av)
                nc.sync.dma_start(
                    out=out_r[t],
                    in_=ov,
                )

            # software-pipelined emission: chunk1's front fills the engine
            # gaps while chunk0's back half drains toward its stores
            front(0)
            back(0, 0)
            front(1)
            back(0, 1)
            back(1, 0)
            back(1, 1)

    nc.compile()
    return nc


def _prep_weights(W_down, b_down, bn1_scale, bn1_bias, bn1_mean, bn1_var,
                  W_conv, bn2_scale, bn2_bias, bn2_mean, bn2_var, W_agg, b_agg):
    f64 = np.float64
    bf16 = ml_dtypes.bfloat16
    a1 = bn1_scale.astype(f64) / np.sqrt(bn1_var.astype(f64) + BN_EPS)
    W1f = W_down.astype(f64) * a1[None, :]                      # [256, 64]
    b1f = (b_down.astype(f64) - bn1_mean) * a1 + bn1_bias       # [64]

    a2 = bn2_scale.astype(f64) / np.sqrt(bn2_var.astype(f64) + BN_EPS)
    Wc = W_conv.astype(f64) * a2[:, None]                       # [64, 128]
    b2f = bn2_bias.astype(f64) - bn2_mean * a2                  # [64]
    W1, W2 = Wc[:, :CI], Wc[:, CI:]
    Wp = W1 - W2

    GS = [len(g) for g in GROUPS]
    # w1 sbuf layout: [128, 2*CI], col-block ch = W1f[ch*128:(ch+1)*128, :]
    w1 = np.concatenate([W1f[:128, :], W1f[128:, :]], axis=1).astype(bf16)
    b1 = np.tile(b1f.reshape(CI, 1), (2, 1)).astype(np.float32)  # [128, 1]

    # wpe: [64, 10*64]: blocks 0..4 = Wp.T/|g_i|, 5..9 = W2.T/|g_j|
    blocks = [Wp.T / GS[i] for i in range(G)] + [W2.T / GS[j] for j in range(G)]
    wpe = np.concatenate(blocks, axis=1).astype(bf16)
    b2 = np.tile(b2f.reshape(CI, 1), (2, 1)).astype(np.float32)

    # wag carries the 1/2 of sigmoid(z) = 0.5*tanh(z/2)+0.5
    wag = (0.5 * np.concatenate(
        [W_agg.astype(f64), b_agg.astype(f64)[None, :]], axis=0
    )).astype(bf16)                                             # [65, 256]
    return w1, b1, wpe, b2, wag


_NC_CACHE = {}


def _run(inputs, trace=False, trace_kwargs=None):
    x = np.ascontiguousarray(np.asarray(inputs["x_bk_c"], dtype=np.float32))
    assert x.shape == (B * K, C), x.shape
    w1, b1, wpe, b2, wag = _prep_weights(
        np.asarray(inputs["W_down"]), np.asarray(inputs["b_down"]),
        np.asarray(inputs["bn1_scale"]), np.asarray(inputs["bn1_bias"]),
        np.asarray(inputs["bn1_mean"]), np.asarray(inputs["bn1_var"]),
        np.asarray(inputs["W_conv"]),
        np.asarray(inputs["bn2_scale"]), np.asarray(inputs["bn2_bias"]),
        np.asarray(inputs["bn2_mean"]), np.asarray(inputs["bn2_var"]),
        np.asarray(inputs["W_agg"]), np.asarray(inputs["b_agg"]),
    )

    if "nc" not in _NC_CACHE:
        _NC_CACHE["nc"] = build_nc()
    nc = _NC_CACHE["nc"]

    in_maps = []
    for c in range(NCORES):
        in_maps.append({
            "x": np.ascontiguousarray(x[c * R:(c + 1) * R]),
            "w1": w1, "b1": b1, "wpe": wpe, "b2": b2, "wag": wag,
            "ident": np.eye(128, dtype=np.float32),
            "ones": np.ones((1, G * BC), dtype=ml_dtypes.bfloat16),
        })
    kw = {}
    if trace:
        kw["trace"] = True
        if trace_kwargs:
            kw["trace_kwargs"] = trace_kwargs
    res = run_bass_kernel_spmd(nc, in_maps, core_ids=list(range(NCORES)), **kw)
    out = np.concatenate([r["out"] for r in res.results], axis=0)
    return out, res


def kernel(**inputs) -> np.ndarray:
    out, _ = _run(inputs)
    return out
